# revision 5
# baseline (speedup 1.0000x reference)
"""Bayesian transformer block on 8 trn2 cores.

Sharding: core c -> batch b=c//2, half h=c%2. Each core handles 8 query
chunks of 256 rows: chunk p = rows [512p+256h, 512p+256h+256) of batch b.
Static per-chunk key extent 512(p+1); the causal boundary is enforced by a
data-driven multiplicative mask on the trailing 4 key tiles.

All matmuls run in float32r (tf32-like, full PE rate); residual adds in
fp32. Attention uses transposed scores S^T[k,q] so the exp output feeds
P@V directly as the moving operand; row sums come from an all-ones
stationary matmul; normalization is a reciprocal broadcast multiply
before the output projection.
"""
import sys, os

for _p in ("/opt/trn_rl_repo", "/root/.axon_site/_ro/trn_rl_repo"):
    if os.path.isdir(_p) and _p not in sys.path:
        sys.path.insert(0, _p)

import numpy as np
from contextlib import ExitStack

import concourse.bass as bass
import concourse.bacc as bacc
import concourse.mybir as mybir
import concourse.tile as tile
from concourse.bass_utils import run_bass_kernel_spmd
from concourse.masks import make_identity

F32 = mybir.dt.float32
F32R = mybir.dt.float32r
AF = mybir.ActivationFunctionType
OP = mybir.AluOpType

DIM = 512
HID = 2048
BS, SLEN = 4, 4096
NCHUNK = 8          # query chunks per core, 256 rows each
QC = 256            # queries per chunk
NQROWS = NCHUNK * QC
KCACHE_GROUPS = 3   # 512-key groups kept resident in SBUF
NG = SLEN // 512
INV_SQRT_D = float(1.0 / np.sqrt(DIM))

_CACHE = {}


def _build_nc():
    nc = bacc.Bacc("TRN2", target_bir_lowering=False, debug=False, num_devices=8,
                   dynamic_dma_scratch_size=2048)

    xf = nc.dram_tensor("xf", [SLEN, DIM], F32, kind="ExternalInput").ap()
    xq = nc.dram_tensor("xq", [NQROWS, DIM], F32, kind="ExternalInput").ap()
    cmask = nc.dram_tensor("cmask", [4, 128, QC], F32, kind="ExternalInput").ap()
    wio = {}
    for w, (o, i) in (("wk", (DIM, DIM)), ("wv", (DIM, DIM)), ("wo", (DIM, DIM)),
                      ("w1", (HID, DIM)), ("w2", (DIM, HID))):
        for sfx in ("mu", "ls", "eps"):
            wio[f"{w}_{sfx}"] = nc.dram_tensor(f"{w}_{sfx}", [o, i], F32,
                                               kind="ExternalInput").ap()
    out = nc.dram_tensor("out", [NQROWS, DIM], F32, kind="ExternalOutput").ap()

    kt_dram = nc.dram_tensor("kt_spill", [DIM, SLEN], F32R).ap()
    v_dram = nc.dram_tensor("v_spill", [SLEN, DIM], F32R).ap()

    with tile.TileContext(nc) as tc, ExitStack() as ctx:
        const = ctx.enter_context(tc.tile_pool(name="const", bufs=1))
        wres = ctx.enter_context(tc.tile_pool(name="wres", bufs=1))
        kvcache = ctx.enter_context(tc.tile_pool(name="kvcache", bufs=1))

        ident = const.tile([128, 128], F32, tag="ident")
        make_identity(nc, ident[:])
        ones32 = const.tile([128, 128], F32, tag="ones32")
        nc.gpsimd.memset(ones32[:], 1.0)
        ones = const.tile([128, 128], F32R, tag="ones")
        nc.vector.tensor_copy(ones[:], ones32[:])
        cm = const.tile([128, 4, QC], F32, tag="cm")
        nc.sync.dma_start(cm[:], cmask.rearrange("j p q -> p j q"))

        def build_wT(w, o_dim, i_dim, dst_pool):
            """Perturb W = mu + exp(ls)*eps, return W^T as i_dim//128 tiles
            [128, o_dim] f32r (partition = input dim)."""
            wt = [dst_pool.tile([128, o_dim], F32R, tag=f"{w}T{i}", name=f"{w}T{i}")
                  for i in range(i_dim // 128)]
            mu_r = wio[f"{w}_mu"].rearrange("(a p) i -> a p i", p=128)
            ls_r = wio[f"{w}_ls"].rearrange("(a p) i -> a p i", p=128)
            ep_r = wio[f"{w}_eps"].rearrange("(a p) i -> a p i", p=128)
            IC = min(i_dim, 512)
            with ExitStack() as stk:
                stage = stk.enter_context(tc.tile_pool(name=f"stg_{w}", bufs=2))
                pst = stk.enter_context(
                    tc.tile_pool(name=f"pst_{w}", bufs=2, space="PSUM"))
                for a in range(o_dim // 128):
                    for cb in range(i_dim // IC):
                        mu = stage.tile([128, IC], F32, tag="mu")
                        ls = stage.tile([128, IC], F32, tag="ls")
                        ep = stage.tile([128, IC], F32, tag="ep")
                        nc.sync.dma_start(mu[:], mu_r[a][:, bass.ts(cb, IC)])
                        nc.sync.dma_start(ls[:], ls_r[a][:, bass.ts(cb, IC)])
                        nc.sync.dma_start(ep[:], ep_r[a][:, bass.ts(cb, IC)])
                        els = stage.tile([128, IC], F32, tag="els")
                        nc.scalar.activation(els[:], ls[:], AF.Exp)
                        prod = stage.tile([128, IC], F32, tag="prod")
                        nc.gpsimd.tensor_tensor(prod[:], els[:], ep[:], op=OP.mult)
                        wnat = stage.tile([128, IC], F32, tag="wnat")
                        nc.vector.tensor_tensor(wnat[:], prod[:], mu[:], op=OP.add)
                        for ii in range(IC // 128):
                            i = cb * (IC // 128) + ii
                            ps = pst.tile([128, 128], F32, tag="tp")
                            nc.tensor.transpose(ps[:], wnat[:, bass.ts(ii, 128)],
                                                ident[:])
                            nc.vector.tensor_copy(wt[i][:, bass.ts(a, 128)], ps[:])
            return wt

        # ---- K^T / V over all 4096 keys (cache 3 groups, spill the rest) ----
        xf_r = xf.rearrange("(g j p) d -> g j p d", j=4, p=128)
        kt_dram_r = kt_dram.rearrange("(t p) s -> t p s", p=128)
        v_dram_r = v_dram.rearrange("(g j p) d -> g j p d", j=4, p=128)

        ktc = [[kvcache.tile([128, 512], F32R, tag=f"ktc{g}_{i}", name=f"ktc{g}_{i}")
                for i in range(4)] for g in range(KCACHE_GROUPS)]
        vc = [[kvcache.tile([128, 512], F32R, tag=f"vc{g}_{j}", name=f"vc{g}_{j}")
               for j in range(4)] for g in range(KCACHE_GROUPS)]

        with ExitStack() as stk:
            wkv = stk.enter_context(tc.tile_pool(name="wkv", bufs=1))
            wkT = build_wT("wk", DIM, DIM, wkv)
            wvT = build_wT("wv", DIM, DIM, wkv)
            stage = stk.enter_context(tc.tile_pool(name="stg_x", bufs=3))
            spill = stk.enter_context(tc.tile_pool(name="spill", bufs=3))
            pstx = stk.enter_context(tc.tile_pool(name="pst_x", bufs=2, space="PSUM"))
            psb = stk.enter_context(tc.tile_pool(name="psB", bufs=2, space="PSUM"))
            for g in range(NG):
                xfT = [stage.tile([128, 512], F32R, tag=f"xfT{i}", name=f"xfT{i}") for i in range(4)]
                for j in range(4):
                    xt = stage.tile([128, DIM], F32, tag="xrow")
                    nc.sync.dma_start(xt[:], xf_r[g, j])
                    for i in range(4):
                        ps = pstx.tile([128, 128], F32, tag="tp")
                        nc.tensor.transpose(ps[:], xt[:, bass.ts(i, 128)], ident[:])
                        nc.vector.tensor_copy(xfT[i][:, bass.ts(j, 128)], ps[:])
                for o in range(4):
                    ps = psb.tile([128, 512], F32, tag="kps")
                    for i in range(4):
                        nc.tensor.matmul(ps[:], wkT[i][:, bass.ts(o, 128)], xfT[i][:],
                                         start=(i == 0), stop=(i == 3))
                    if g < KCACHE_GROUPS:
                        nc.scalar.copy(ktc[g][o][:], ps[:])
                    else:
                        sp = spill.tile([128, 512], F32R, tag="ksp")
                        nc.scalar.copy(sp[:], ps[:])
                        nc.sync.dma_start(kt_dram_r[o, :, bass.ts(g, 512)], sp[:])
                for j in range(4):
                    ps = psb.tile([128, 512], F32, tag="vps")
                    for i in range(4):
                        nc.tensor.matmul(ps[:], xfT[i][:, bass.ts(j, 128)], wvT[i][:],
                                         start=(i == 0), stop=(i == 3))
                    if g < KCACHE_GROUPS:
                        nc.vector.tensor_copy(vc[g][j][:], ps[:])
                    else:
                        sp = spill.tile([128, 512], F32R, tag="vsp")
                        nc.vector.tensor_copy(sp[:], ps[:])
                        nc.sync.dma_start(v_dram_r[g, j], sp[:])

        woT = build_wT("wo", DIM, DIM, wres)
        w1T = build_wT("w1", HID, DIM, wres)
        w2T = build_wT("w2", DIM, HID, wres)

        # ---- per-slot attention + FFN ----
        slot = ctx.enter_context(tc.tile_pool(name="slot", bufs=2))
        s1 = ctx.enter_context(tc.tile_pool(name="s1", bufs=1))
        pt_pool = ctx.enter_context(tc.tile_pool(name="pt", bufs=4))
        ff_pool = ctx.enter_context(tc.tile_pool(name="ff", bufs=3))
        kstream = ctx.enter_context(tc.tile_pool(name="kstream", bufs=2))
        # PSUM budget (8 banks): psS 2 ("sT": transposes+scores+ff1) +
        # psH 4 (hT0-3, reused by woT/woN) + psE 2 ("ff2": s_rep, ff2_0/1)
        psS = ctx.enter_context(tc.tile_pool(name="psS", bufs=2, space="PSUM"))
        psH = ctx.enter_context(tc.tile_pool(name="psH", bufs=1, space="PSUM"))
        psE = ctx.enter_context(tc.tile_pool(name="psE", bufs=2, space="PSUM"))

        xq_r = xq.rearrange("(p j q) d -> p j q d", j=2, q=128)
        out_r = out.rearrange("(p j q) d -> p j q d", j=2, q=128)

        for p in range(NCHUNK):
            xq_nat = [slot.tile([128, DIM], F32, tag=f"xqn{j}", name=f"xqn{j}") for j in range(2)]
            xqT = [s1.tile([128, QC], F32R, tag=f"xqT{i}", name=f"xqT{i}", bufs=2) for i in range(4)]
            for j in range(2):
                nc.sync.dma_start(xq_nat[j][:], xq_r[p, j])
                for i in range(4):
                    ps = psS.tile([128, 128], F32, tag="sT")
                    nc.tensor.transpose(ps[:], xq_nat[j][:, bass.ts(i, 128)], ident[:])
                    nc.vector.tensor_copy(xqT[i][:, bass.ts(j, 128)], ps[:])

            hT = [psH.tile([128, QC], F32, tag=f"hT{i}", name=f"hT{i}") for i in range(4)]
            s_rep = psE.tile([128, QC], F32, tag="ff2")
            first_av = True
            for g in range(p + 1):
                if g < KCACHE_GROUPS:
                    kt_g, v_g = ktc[g], vc[g]
                else:
                    kt_g = [kstream.tile([128, 512], F32R, tag=f"kts{i}", name=f"kts{i}")
                            for i in range(4)]
                    v_g = [kstream.tile([128, 512], F32R, tag=f"vs{j}", name=f"vs{j}")
                           for j in range(4)]
                    for i in range(4):
                        nc.sync.dma_start(kt_g[i][:], kt_dram_r[i, :, bass.ts(g, 512)])
                    for j in range(4):
                        nc.sync.dma_start(v_g[j][:], v_dram_r[g, j])
                diag = (g == p)
                for j in range(4):
                    ps = psS.tile([128, QC], F32, tag="sT")
                    for i in range(4):
                        nc.tensor.matmul(ps[:], kt_g[i][:, bass.ts(j, 128)], xqT[i][:],
                                         start=(i == 0), stop=(i == 3))
                    pt = pt_pool.tile([128, QC], F32R, tag="pt")
                    if diag:
                        pe = pt_pool.tile([128, QC], F32R, tag="pe", bufs=2)
                        nc.scalar.activation(pe[:], ps[:], AF.Exp, scale=INV_SQRT_D)
                        nc.vector.tensor_tensor(pt[:], pe[:], cm[:, j, :], op=OP.mult)
                    else:
                        nc.scalar.activation(pt[:], ps[:], AF.Exp, scale=INV_SQRT_D)
                    last_av = (g == p) and (j == 3)
                    for i in range(4):
                        nc.tensor.matmul(hT[i][:], v_g[j][:, bass.ts(i, 128)], pt[:],
                                         start=first_av, stop=last_av)
                    nc.tensor.matmul(s_rep[:], ones[:], pt[:],
                                     start=first_av, stop=last_av)
                    first_av = False

            r_bc = slot.tile([128, QC], F32, tag="r_bc")
            nc.vector.reciprocal(r_bc[:], s_rep[:])
            h_nrm = [s1.tile([128, QC], F32R, tag=f"hn{i}", name=f"hn{i}", bufs=2) for i in range(4)]
            for i in range(4):
                nc.vector.tensor_tensor(h_nrm[i][:], hT[i][:], r_bc[:], op=OP.mult)

            h_resT = [s1.tile([128, QC], F32R, tag=f"hrT{o}", name=f"hrT{o}", bufs=2) for o in range(4)]
            for o in range(4):
                ps = psH.tile([128, QC], F32, tag=f"hT{o % 2}")
                for i in range(4):
                    nc.tensor.matmul(ps[:], woT[i][:, bass.ts(o, 128)], h_nrm[i][:],
                                     start=(i == 0), stop=(i == 3))
                nc.vector.tensor_tensor(h_resT[o][:], ps[:], xqT[o][:], op=OP.add)
            h_resN = [slot.tile([128, DIM], F32, tag=f"hrN{j}", name=f"hrN{j}") for j in range(2)]
            for j in range(2):
                ps = psH.tile([128, DIM], F32, tag=f"hT{2 + j}")
                for i in range(4):
                    nc.tensor.matmul(ps[:], h_nrm[i][:, bass.ts(j, 128)], woT[i][:],
                                     start=(i == 0), stop=(i == 3))
                nc.vector.tensor_tensor(h_resN[j][:], ps[:], xq_nat[j][:], op=OP.add)

            ff2 = [psE.tile([128, DIM], F32, tag="ff2", name=f"ff2_{j}") for j in range(2)]
            for hh in range(HID // 128):
                ps = psS.tile([128, QC], F32, tag="sT")
                for i in range(4):
                    nc.tensor.matmul(ps[:], w1T[i][:, bass.ts(hh, 128)], h_resT[i][:],
                                     start=(i == 0), stop=(i == 3))
                f1 = ff_pool.tile([128, QC], F32R, tag="f1")
                nc.scalar.activation(f1[:], ps[:], AF.Relu)
                for j in range(2):
                    nc.tensor.matmul(ff2[j][:], f1[:, bass.ts(j, 128)], w2T[hh][:],
                                     start=(hh == 0), stop=(hh == HID // 128 - 1))
            for j in range(2):
                ot = slot.tile([128, DIM], F32, tag=f"ot{j}")
                nc.vector.tensor_tensor(ot[:], ff2[j][:], h_resN[j][:], op=OP.add)
                nc.sync.dma_start(out_r[p, j], ot[:])

    nc.compile()
    return nc


def _shard_inputs(inputs):
    x = np.ascontiguousarray(inputs["x"], dtype=np.float32)
    in_maps = []
    for c in range(8):
        b, h = c // 2, c % 2
        xb = np.ascontiguousarray(x[b])
        xq = np.ascontiguousarray(
            xb.reshape(NCHUNK, 512, DIM)[:, QC * h:QC * h + QC, :].reshape(NQROWS, DIM))
        kr = np.arange(128)[None, :, None]
        qr = np.arange(QC)[None, None, :]
        jj = np.arange(4)[:, None, None]
        cmsk = (qr >= kr + 128 * jj - QC * h).astype(np.float32)
        m = {"xf": xb, "xq": xq, "cmask": np.ascontiguousarray(cmsk)}
        for k, v in inputs.items():
            if k not in ("x", "mask"):
                m[k] = np.ascontiguousarray(v, dtype=np.float32)
        in_maps.append(m)
    return in_maps


def kernel(**inputs):
    if "nc" not in _CACHE:
        _CACHE["nc"] = _build_nc()
    nc = _CACHE["nc"]
    in_maps = _shard_inputs(inputs)
    res = run_bass_kernel_spmd(nc, in_maps, core_ids=list(range(8)))
    out = np.empty((BS, SLEN, DIM), dtype=np.float32)
    for c in range(8):
        b, h = c // 2, c % 2
        o = res.results[c]["out"].reshape(NCHUNK, QC, DIM)
        out.reshape(BS, NCHUNK, 512, DIM)[b, :, QC * h:QC * h + QC, :] = o
    return out


# revision 6
# speedup vs baseline: 1.1852x; 1.1852x over previous
"""Bayesian transformer block on 8 trn2 cores.

Sharding: core c -> batch b=c//2, half h=c%2. Each core handles 8 query
chunks of 256 rows: chunk p = rows [512p+256h, 512p+256h+256) of batch b.
Static per-chunk key extent 512(p+1); the causal boundary is enforced by a
data-driven multiplicative mask on the trailing 4 key tiles.

All matmuls run in float32r (tf32-like, full PE rate); residual adds in
fp32. Attention uses transposed scores S^T[k,q] so the exp output feeds
P@V directly as the moving operand; row sums come from an all-ones
stationary matmul; normalization is a reciprocal broadcast multiply
before the output projection.
"""
import sys, os

for _p in ("/opt/trn_rl_repo", "/root/.axon_site/_ro/trn_rl_repo"):
    if os.path.isdir(_p) and _p not in sys.path:
        sys.path.insert(0, _p)

import numpy as np
from contextlib import ExitStack

import concourse.bass as bass
import concourse.bacc as bacc
import concourse.mybir as mybir
import concourse.tile as tile
from concourse.bass_utils import run_bass_kernel_spmd
from concourse.masks import make_identity

F32 = mybir.dt.float32
F32R = mybir.dt.float32r
AF = mybir.ActivationFunctionType
OP = mybir.AluOpType

DIM = 512
HID = 2048
BS, SLEN = 4, 4096
NCHUNK = 8          # query chunks per core, 256 rows each
QC = 256            # queries per chunk
NQROWS = NCHUNK * QC
KCACHE_GROUPS = 3   # 512-key groups kept resident in SBUF
NG = SLEN // 512
INV_SQRT_D = float(1.0 / np.sqrt(DIM))

_CACHE = {}


def _build_nc(reps=1):
    nc = bacc.Bacc("TRN2", target_bir_lowering=False, debug=False, num_devices=8,
                   dynamic_dma_scratch_size=2048)

    xf = nc.dram_tensor("xf", [SLEN, DIM], F32, kind="ExternalInput").ap()
    xq = nc.dram_tensor("xq", [NQROWS, DIM], F32, kind="ExternalInput").ap()
    cmask = nc.dram_tensor("cmask", [4, 128, QC], F32, kind="ExternalInput").ap()
    wio = {}
    for w, (o, i) in (("wk", (DIM, DIM)), ("wv", (DIM, DIM)), ("wo", (DIM, DIM)),
                      ("w1", (HID, DIM)), ("w2", (DIM, HID))):
        for sfx in ("mu", "ls", "eps"):
            wio[f"{w}_{sfx}"] = nc.dram_tensor(f"{w}_{sfx}", [o, i], F32,
                                               kind="ExternalInput").ap()
    out = nc.dram_tensor("out", [NQROWS, DIM], F32, kind="ExternalOutput").ap()

    kt_dram = nc.dram_tensor("kt_spill", [DIM, SLEN], F32R).ap()
    v_dram = nc.dram_tensor("v_spill", [SLEN, DIM], F32R).ap()

    with tile.TileContext(nc) as tc:
      for _rep in range(reps):
       with ExitStack() as ctx:
        P = lambda n: f"{n}_{_rep}"
        const = ctx.enter_context(tc.tile_pool(name=P("const"), bufs=1))
        wres = ctx.enter_context(tc.tile_pool(name=P("wres"), bufs=1))
        kvcache = ctx.enter_context(tc.tile_pool(name=P("kvcache"), bufs=1))

        ident = const.tile([128, 128], F32, tag="ident")
        make_identity(nc, ident[:])
        ones32 = const.tile([128, 128], F32, tag="ones32")
        nc.gpsimd.memset(ones32[:], 1.0)
        ones = const.tile([128, 128], F32R, tag="ones")
        nc.vector.tensor_copy(ones[:], ones32[:])
        cm = const.tile([128, 4, QC], F32, tag="cm")
        nc.sync.dma_start(cm[:], cmask.rearrange("j p q -> p j q"))

        def build_wT(w, o_dim, i_dim, dst_pool):
            """Perturb W = mu + exp(ls)*eps, return W^T as i_dim//128 tiles
            [128, o_dim] f32r (partition = input dim)."""
            wt = [dst_pool.tile([128, o_dim], F32R, tag=f"{w}T{i}", name=f"{w}T{i}")
                  for i in range(i_dim // 128)]
            mu_r = wio[f"{w}_mu"].rearrange("(a p) i -> a p i", p=128)
            ls_r = wio[f"{w}_ls"].rearrange("(a p) i -> a p i", p=128)
            ep_r = wio[f"{w}_eps"].rearrange("(a p) i -> a p i", p=128)
            IC = min(i_dim, 512)
            with ExitStack() as stk:
                stage = stk.enter_context(tc.tile_pool(name=P(f"stg_{w}"), bufs=2))
                pst = stk.enter_context(
                    tc.tile_pool(name=P(f"pst_{w}"), bufs=2, space="PSUM"))
                for a in range(o_dim // 128):
                    for cb in range(i_dim // IC):
                        mu = stage.tile([128, IC], F32, tag="mu")
                        ls = stage.tile([128, IC], F32, tag="ls")
                        ep = stage.tile([128, IC], F32, tag="ep")
                        nc.sync.dma_start(mu[:], mu_r[a][:, bass.ts(cb, IC)])
                        nc.sync.dma_start(ls[:], ls_r[a][:, bass.ts(cb, IC)])
                        nc.sync.dma_start(ep[:], ep_r[a][:, bass.ts(cb, IC)])
                        els = stage.tile([128, IC], F32, tag="els")
                        nc.scalar.activation(els[:], ls[:], AF.Exp)
                        prod = stage.tile([128, IC], F32, tag="prod")
                        nc.gpsimd.tensor_tensor(prod[:], els[:], ep[:], op=OP.mult)
                        wnat = stage.tile([128, IC], F32, tag="wnat")
                        nc.vector.tensor_tensor(wnat[:], prod[:], mu[:], op=OP.add)
                        for ii in range(IC // 128):
                            i = cb * (IC // 128) + ii
                            ps = pst.tile([128, 128], F32, tag="tp")
                            nc.tensor.transpose(ps[:], wnat[:, bass.ts(ii, 128)],
                                                ident[:])
                            nc.vector.tensor_copy(wt[i][:, bass.ts(a, 128)], ps[:])
            return wt

        # ---- K^T / V over all 4096 keys (cache 3 groups, spill the rest) ----
        xf_r = xf.rearrange("(g j p) d -> g j p d", j=4, p=128)
        kt_dram_r = kt_dram.rearrange("(t p) s -> t p s", p=128)
        v_dram_r = v_dram.rearrange("(g j p) d -> g j p d", j=4, p=128)

        ktc = [[kvcache.tile([128, 512], F32R, tag=f"ktc{g}_{i}", name=f"ktc{g}_{i}")
                for i in range(4)] for g in range(KCACHE_GROUPS)]
        vc = [[kvcache.tile([128, 512], F32R, tag=f"vc{g}_{j}", name=f"vc{g}_{j}")
               for j in range(4)] for g in range(KCACHE_GROUPS)]

        with ExitStack() as stk:
            wkv = stk.enter_context(tc.tile_pool(name=P("wkv"), bufs=1))
            wkT = build_wT("wk", DIM, DIM, wkv)
            wvT = build_wT("wv", DIM, DIM, wkv)
            stage = stk.enter_context(tc.tile_pool(name=P("stg_x"), bufs=3))
            spill = stk.enter_context(tc.tile_pool(name=P("spill"), bufs=3))
            pstx = stk.enter_context(tc.tile_pool(name=P("pst_x"), bufs=2, space="PSUM"))
            psb = stk.enter_context(tc.tile_pool(name=P("psB"), bufs=2, space="PSUM"))
            for g in range(NG):
                xfT = [stage.tile([128, 512], F32R, tag=f"xfT{i}", name=f"xfT{i}") for i in range(4)]
                for j in range(4):
                    xt = stage.tile([128, DIM], F32, tag="xrow")
                    nc.sync.dma_start(xt[:], xf_r[g, j])
                    for i in range(4):
                        ps = pstx.tile([128, 128], F32, tag="tp")
                        nc.tensor.transpose(ps[:], xt[:, bass.ts(i, 128)], ident[:])
                        nc.vector.tensor_copy(xfT[i][:, bass.ts(j, 128)], ps[:])
                for o in range(4):
                    ps = psb.tile([128, 512], F32, tag="kps")
                    for i in range(4):
                        nc.tensor.matmul(ps[:], wkT[i][:, bass.ts(o, 128)], xfT[i][:],
                                         start=(i == 0), stop=(i == 3))
                    if g < KCACHE_GROUPS:
                        nc.scalar.copy(ktc[g][o][:], ps[:])
                    else:
                        sp = spill.tile([128, 512], F32R, tag="ksp")
                        nc.scalar.copy(sp[:], ps[:])
                        nc.sync.dma_start(kt_dram_r[o, :, bass.ts(g, 512)], sp[:])
                for j in range(4):
                    ps = psb.tile([128, 512], F32, tag="vps")
                    for i in range(4):
                        nc.tensor.matmul(ps[:], xfT[i][:, bass.ts(j, 128)], wvT[i][:],
                                         start=(i == 0), stop=(i == 3))
                    if g < KCACHE_GROUPS:
                        nc.vector.tensor_copy(vc[g][j][:], ps[:])
                    else:
                        sp = spill.tile([128, 512], F32R, tag="vsp")
                        nc.vector.tensor_copy(sp[:], ps[:])
                        nc.sync.dma_start(v_dram_r[g, j], sp[:])

        woT = build_wT("wo", DIM, DIM, wres)
        w1T = build_wT("w1", HID, DIM, wres)
        w2T = build_wT("w2", DIM, HID, wres)

        # ---- per-slot attention + FFN ----
        slot = ctx.enter_context(tc.tile_pool(name=P("slot"), bufs=2))
        s1 = ctx.enter_context(tc.tile_pool(name=P("s1"), bufs=1))
        pt_pool = ctx.enter_context(tc.tile_pool(name=P("pt"), bufs=4))
        ff_pool = ctx.enter_context(tc.tile_pool(name=P("ff"), bufs=3))
        kstream = ctx.enter_context(tc.tile_pool(name=P("kstream"), bufs=2))
        # PSUM budget (8 banks): psS 2 ("sT": transposes+scores+ff1) +
        # psH 4 (hT0-3, reused by woT/woN) + psE 2 ("ff2": s_rep, ff2_0/1)
        psS = ctx.enter_context(tc.tile_pool(name=P("psS"), bufs=2, space="PSUM"))
        psH = ctx.enter_context(tc.tile_pool(name=P("psH"), bufs=1, space="PSUM"))
        psE = ctx.enter_context(tc.tile_pool(name=P("psE"), bufs=2, space="PSUM"))

        xq_r = xq.rearrange("(p j q) d -> p j q d", j=2, q=128)
        out_r = out.rearrange("(p j q) d -> p j q d", j=2, q=128)

        for p in range(NCHUNK):
            xq_nat = [slot.tile([128, DIM], F32, tag=f"xqn{j}", name=f"xqn{j}") for j in range(2)]
            xqT = [s1.tile([128, QC], F32R, tag=f"xqT{i}", name=f"xqT{i}", bufs=2) for i in range(4)]
            for j in range(2):
                nc.sync.dma_start(xq_nat[j][:], xq_r[p, j])
                for i in range(4):
                    ps = psS.tile([128, 128], F32, tag="sT")
                    nc.tensor.transpose(ps[:], xq_nat[j][:, bass.ts(i, 128)], ident[:])
                    nc.vector.tensor_copy(xqT[i][:, bass.ts(j, 128)], ps[:])

            hT = [psH.tile([128, QC], F32, tag=f"hT{i}", name=f"hT{i}") for i in range(4)]
            s_rep = psE.tile([128, QC], F32, tag="ff2")
            first_av = True
            for g in range(p + 1):
                if g < KCACHE_GROUPS:
                    kt_g, v_g = ktc[g], vc[g]
                else:
                    kt_g = [kstream.tile([128, 512], F32R, tag=f"kts{i}", name=f"kts{i}")
                            for i in range(4)]
                    v_g = [kstream.tile([128, 512], F32R, tag=f"vs{j}", name=f"vs{j}")
                           for j in range(4)]
                    for i in range(4):
                        nc.sync.dma_start(kt_g[i][:], kt_dram_r[i, :, bass.ts(g, 512)])
                    for j in range(4):
                        nc.sync.dma_start(v_g[j][:], v_dram_r[g, j])
                diag = (g == p)
                for j in range(4):
                    ps = psS.tile([128, QC], F32, tag="sT")
                    for i in range(4):
                        nc.tensor.matmul(ps[:], kt_g[i][:, bass.ts(j, 128)], xqT[i][:],
                                         start=(i == 0), stop=(i == 3))
                    pt = pt_pool.tile([128, QC], F32R, tag="pt")
                    if diag:
                        pe = pt_pool.tile([128, QC], F32R, tag="pe", bufs=2)
                        nc.scalar.activation(pe[:], ps[:], AF.Exp, scale=INV_SQRT_D)
                        nc.vector.tensor_tensor(pt[:], pe[:], cm[:, j, :], op=OP.mult)
                    else:
                        nc.scalar.activation(pt[:], ps[:], AF.Exp, scale=INV_SQRT_D)
                    last_av = (g == p) and (j == 3)
                    for i in range(4):
                        nc.tensor.matmul(hT[i][:], v_g[j][:, bass.ts(i, 128)], pt[:],
                                         start=first_av, stop=last_av)
                    nc.tensor.matmul(s_rep[:], ones[:], pt[:],
                                     start=first_av, stop=last_av)
                    first_av = False

            r_bc = slot.tile([128, QC], F32, tag="r_bc")
            nc.vector.reciprocal(r_bc[:], s_rep[:])
            h_nrm = [s1.tile([128, QC], F32R, tag=f"hn{i}", name=f"hn{i}", bufs=2) for i in range(4)]
            for i in range(4):
                nc.vector.tensor_tensor(h_nrm[i][:], hT[i][:], r_bc[:], op=OP.mult)

            h_resT = [s1.tile([128, QC], F32R, tag=f"hrT{o}", name=f"hrT{o}", bufs=2) for o in range(4)]
            for o in range(4):
                ps = psH.tile([128, QC], F32, tag=f"hT{o % 2}")
                for i in range(4):
                    nc.tensor.matmul(ps[:], woT[i][:, bass.ts(o, 128)], h_nrm[i][:],
                                     start=(i == 0), stop=(i == 3))
                nc.vector.tensor_tensor(h_resT[o][:], ps[:], xqT[o][:], op=OP.add)
            h_resN = [slot.tile([128, DIM], F32, tag=f"hrN{j}", name=f"hrN{j}") for j in range(2)]
            for j in range(2):
                ps = psH.tile([128, DIM], F32, tag=f"hT{2 + j}")
                for i in range(4):
                    nc.tensor.matmul(ps[:], h_nrm[i][:, bass.ts(j, 128)], woT[i][:],
                                     start=(i == 0), stop=(i == 3))
                nc.vector.tensor_tensor(h_resN[j][:], ps[:], xq_nat[j][:], op=OP.add)

            ff2 = [psE.tile([128, DIM], F32, tag="ff2", name=f"ff2_{j}") for j in range(2)]
            for hh in range(HID // 128):
                ps = psS.tile([128, QC], F32, tag="sT")
                for i in range(4):
                    nc.tensor.matmul(ps[:], w1T[i][:, bass.ts(hh, 128)], h_resT[i][:],
                                     start=(i == 0), stop=(i == 3))
                f1 = ff_pool.tile([128, QC], F32R, tag="f1")
                nc.scalar.activation(f1[:], ps[:], AF.Relu)
                for j in range(2):
                    nc.tensor.matmul(ff2[j][:], f1[:, bass.ts(j, 128)], w2T[hh][:],
                                     start=(hh == 0), stop=(hh == HID // 128 - 1))
            for j in range(2):
                ot = slot.tile([128, DIM], F32, tag=f"ot{j}")
                nc.vector.tensor_tensor(ot[:], ff2[j][:], h_resN[j][:], op=OP.add)
                nc.sync.dma_start(out_r[p, j], ot[:])

    nc.compile()
    return nc


def _shard_inputs(inputs):
    x = np.ascontiguousarray(inputs["x"], dtype=np.float32)
    in_maps = []
    for c in range(8):
        b, h = c // 2, c % 2
        xb = np.ascontiguousarray(x[b])
        xq = np.ascontiguousarray(
            xb.reshape(NCHUNK, 512, DIM)[:, QC * h:QC * h + QC, :].reshape(NQROWS, DIM))
        kr = np.arange(128)[None, :, None]
        qr = np.arange(QC)[None, None, :]
        jj = np.arange(4)[:, None, None]
        cmsk = (qr >= kr + 128 * jj - QC * h).astype(np.float32)
        m = {"xf": xb, "xq": xq, "cmask": np.ascontiguousarray(cmsk)}
        for k, v in inputs.items():
            if k not in ("x", "mask"):
                m[k] = np.ascontiguousarray(v, dtype=np.float32)
        in_maps.append(m)
    return in_maps


def kernel(**inputs):
    if "nc" not in _CACHE:
        _CACHE["nc"] = _build_nc()
    nc = _CACHE["nc"]
    in_maps = _shard_inputs(inputs)
    res = run_bass_kernel_spmd(nc, in_maps, core_ids=list(range(8)))
    out = np.empty((BS, SLEN, DIM), dtype=np.float32)
    for c in range(8):
        b, h = c // 2, c % 2
        o = res.results[c]["out"].reshape(NCHUNK, QC, DIM)
        out.reshape(BS, NCHUNK, 512, DIM)[b, :, QC * h:QC * h + QC, :] = o
    return out


# revision 8
# speedup vs baseline: 1.8983x; 1.6017x over previous
"""Bayesian transformer block on 8 trn2 cores.

Sharding: core c -> batch b=c//2, half h=c%2. Each core handles 8 query
chunks of 256 rows: chunk p = rows [512p+256h, 512p+256h+256) of batch b.
Static per-chunk key extent 512(p+1); the causal boundary is enforced by a
data-driven multiplicative mask on the trailing 4 key tiles.

All matmuls run in float32r (tf32-like, full PE rate); residual adds in
fp32. Attention uses transposed scores S^T[k,q] so the exp output feeds
P@V directly as the moving operand; row sums come from an all-ones
stationary matmul; normalization is a reciprocal broadcast multiply
before the output projection.
"""
import sys, os

for _p in ("/opt/trn_rl_repo", "/root/.axon_site/_ro/trn_rl_repo"):
    if os.path.isdir(_p) and _p not in sys.path:
        sys.path.insert(0, _p)

import numpy as np
from contextlib import ExitStack

import concourse.bass as bass
import concourse.bacc as bacc
import concourse.mybir as mybir
import concourse.tile as tile
from concourse.bass_utils import run_bass_kernel_spmd
from concourse.masks import make_identity

F32 = mybir.dt.float32
F32R = mybir.dt.float32r
AF = mybir.ActivationFunctionType
OP = mybir.AluOpType

DIM = 512
HID = 2048
BS, SLEN = 4, 4096
NCHUNK = 8          # query chunks per core, 256 rows each
QC = 256            # queries per chunk
NQROWS = NCHUNK * QC
KCACHE_GROUPS = 3   # 512-key groups kept resident in SBUF
NG = SLEN // 512
INV_SQRT_D = float(1.0 / np.sqrt(DIM))

_CACHE = {}


def _build_nc(reps=1):
    nc = bacc.Bacc("TRN2", target_bir_lowering=False, debug=False, num_devices=8,
                   dynamic_dma_scratch_size=2048)

    xf = nc.dram_tensor("xf", [SLEN, DIM], F32, kind="ExternalInput").ap()
    xq = nc.dram_tensor("xq", [NQROWS, DIM], F32, kind="ExternalInput").ap()
    cmask = nc.dram_tensor("cmask", [4, 128, QC], F32, kind="ExternalInput").ap()
    wio = {}
    for w, (o, i) in (("wk", (DIM, DIM)), ("wv", (DIM, DIM)), ("wo", (DIM, DIM)),
                      ("w1", (HID, DIM)), ("w2", (DIM, HID))):
        for sfx in ("mu", "ls", "eps"):
            wio[f"{w}_{sfx}"] = nc.dram_tensor(f"{w}_{sfx}", [o, i], F32,
                                               kind="ExternalInput").ap()
    out = nc.dram_tensor("out", [NQROWS, DIM], F32, kind="ExternalOutput").ap()

    kt_dram = nc.dram_tensor("kt_spill", [DIM, SLEN], F32R).ap()
    v_dram = nc.dram_tensor("v_spill", [SLEN, DIM], F32R).ap()

    with tile.TileContext(nc) as tc:
      for _rep in range(reps):
       with ExitStack() as ctx:
        P = lambda n: f"{n}_{_rep}"
        const = ctx.enter_context(tc.tile_pool(name=P("const"), bufs=1))
        wres = ctx.enter_context(tc.tile_pool(name=P("wres"), bufs=1))
        kvcache = ctx.enter_context(tc.tile_pool(name=P("kvcache"), bufs=1))

        ident = const.tile([128, 128], F32, tag="ident")
        make_identity(nc, ident[:])
        ones32 = const.tile([128, 128], F32, tag="ones32")
        nc.gpsimd.memset(ones32[:], 1.0)
        ones = const.tile([128, 128], F32R, tag="ones")
        nc.vector.tensor_copy(ones[:], ones32[:])
        ident_r = const.tile([128, 128], F32R, tag="ident_r")
        nc.vector.tensor_copy(ident_r[:], ident[:])
        cm = const.tile([128, 4, QC], F32, tag="cm")
        nc.sync.dma_start(cm[:], cmask.rearrange("j p q -> p j q"))

        def build_wT(w, o_dim, i_dim, dst_pool):
            """Perturb W = mu + exp(ls)*eps, return W^T as i_dim//128 tiles
            [128, o_dim] f32r (partition = input dim)."""
            wt = [dst_pool.tile([128, o_dim], F32R, tag=f"{w}T{i}", name=f"{w}T{i}")
                  for i in range(i_dim // 128)]
            mu_r = wio[f"{w}_mu"].rearrange("(a p) i -> a p i", p=128)
            ls_r = wio[f"{w}_ls"].rearrange("(a p) i -> a p i", p=128)
            ep_r = wio[f"{w}_eps"].rearrange("(a p) i -> a p i", p=128)
            IC = min(i_dim, 512)
            with ExitStack() as stk:
                stage = stk.enter_context(tc.tile_pool(name=P(f"stg_{w}"), bufs=2))
                pst = stk.enter_context(
                    tc.tile_pool(name=P(f"pst_{w}"), bufs=2, space="PSUM"))
                for a in range(o_dim // 128):
                    for cb in range(i_dim // IC):
                        mu = stage.tile([128, IC], F32, tag="mu")
                        ls = stage.tile([128, IC], F32, tag="ls")
                        ep = stage.tile([128, IC], F32, tag="ep")
                        nc.sync.dma_start(mu[:], mu_r[a][:, bass.ts(cb, IC)])
                        nc.sync.dma_start(ls[:], ls_r[a][:, bass.ts(cb, IC)])
                        nc.sync.dma_start(ep[:], ep_r[a][:, bass.ts(cb, IC)])
                        els = stage.tile([128, IC], F32, tag="els")
                        nc.scalar.activation(els[:], ls[:], AF.Exp)
                        prod = stage.tile([128, IC], F32, tag="prod")
                        nc.gpsimd.tensor_tensor(prod[:], els[:], ep[:], op=OP.mult)
                        wnat = stage.tile([128, IC], F32, tag="wnat")
                        nc.vector.tensor_tensor(wnat[:], prod[:], mu[:], op=OP.add)
                        for ii in range(IC // 128):
                            i = cb * (IC // 128) + ii
                            ps = pst.tile([128, 128], F32, tag="tp")
                            nc.tensor.transpose(ps[:], wnat[:, bass.ts(ii, 128)],
                                                ident[:])
                            nc.vector.tensor_copy(wt[i][:, bass.ts(a, 128)], ps[:])
            return wt

        # ---- K^T / V over all 4096 keys (cache 3 groups, spill the rest) ----
        xf_r = xf.rearrange("(g j p) d -> g j p d", j=4, p=128)
        kt_dram_r = kt_dram.rearrange("(t p) s -> t p s", p=128)
        v_dram_r = v_dram.rearrange("(g j p) d -> g j p d", j=4, p=128)

        ktc = [[kvcache.tile([128, 512], F32R, tag=f"ktc{g}_{i}", name=f"ktc{g}_{i}")
                for i in range(4)] for g in range(KCACHE_GROUPS)]
        vc = [[kvcache.tile([128, 512], F32R, tag=f"vc{g}_{j}", name=f"vc{g}_{j}")
               for j in range(4)] for g in range(KCACHE_GROUPS)]

        with ExitStack() as stk:
            wkv = stk.enter_context(tc.tile_pool(name=P("wkv"), bufs=1))
            wkT = build_wT("wk", DIM, DIM, wkv)
            wvT = build_wT("wv", DIM, DIM, wkv)
            stage = stk.enter_context(tc.tile_pool(name=P("stg_x"), bufs=3))
            spill = stk.enter_context(tc.tile_pool(name=P("spill"), bufs=3))
            pstx = stk.enter_context(tc.tile_pool(name=P("pst_x"), bufs=2, space="PSUM"))
            psb = stk.enter_context(tc.tile_pool(name=P("psB"), bufs=2, space="PSUM"))
            for g in range(NG):
                xfT = [stage.tile([128, 512], F32R, tag=f"xfT{i}", name=f"xfT{i}") for i in range(4)]
                for j in range(4):
                    xt = stage.tile([128, DIM], F32, tag="xrow")
                    nc.sync.dma_start(xt[:], xf_r[g, j])
                    for i in range(4):
                        ps = pstx.tile([128, 128], F32, tag="tp")
                        nc.tensor.transpose(ps[:], xt[:, bass.ts(i, 128)], ident[:])
                        nc.vector.tensor_copy(xfT[i][:, bass.ts(j, 128)], ps[:])
                for o in range(4):
                    ps = psb.tile([128, 512], F32, tag="kps")
                    for i in range(4):
                        nc.tensor.matmul(ps[:], wkT[i][:, bass.ts(o, 128)], xfT[i][:],
                                         start=(i == 0), stop=(i == 3))
                    if g < KCACHE_GROUPS:
                        nc.scalar.copy(ktc[g][o][:], ps[:])
                    else:
                        sp = spill.tile([128, 512], F32R, tag="ksp")
                        nc.scalar.copy(sp[:], ps[:])
                        nc.sync.dma_start(kt_dram_r[o, :, bass.ts(g, 512)], sp[:])
                for j in range(4):
                    ps = psb.tile([128, 512], F32, tag="vps")
                    for i in range(4):
                        nc.tensor.matmul(ps[:], xfT[i][:, bass.ts(j, 128)], wvT[i][:],
                                         start=(i == 0), stop=(i == 3))
                    if g < KCACHE_GROUPS:
                        nc.vector.tensor_copy(vc[g][j][:], ps[:])
                    else:
                        sp = spill.tile([128, 512], F32R, tag="vsp")
                        nc.vector.tensor_copy(sp[:], ps[:])
                        nc.sync.dma_start(v_dram_r[g, j], sp[:])

        woT = build_wT("wo", DIM, DIM, wres)
        w1T = build_wT("w1", HID, DIM, wres)
        w2T = build_wT("w2", DIM, HID, wres)

        # ---- per-slot attention + FFN ----
        slot = ctx.enter_context(tc.tile_pool(name=P("slot"), bufs=2))
        s1 = ctx.enter_context(tc.tile_pool(name=P("s1"), bufs=1))
        pt_pool = ctx.enter_context(tc.tile_pool(name=P("pt"), bufs=4))
        ff_pool = ctx.enter_context(tc.tile_pool(name=P("ff"), bufs=3))
        kstream = ctx.enter_context(tc.tile_pool(name=P("kstream"), bufs=2))
        # PSUM budget (8 banks): psS 2 ("sT": transposes+scores) +
        # psH 2 (packed AV accumulators) + psE 2 ("ff2": s_rep & ff2_0/1) +
        # psW 2 ("wf": woT then ff1)
        psS = ctx.enter_context(tc.tile_pool(name=P("psS"), bufs=2, space="PSUM"))
        psH = ctx.enter_context(tc.tile_pool(name=P("psH"), bufs=1, space="PSUM"))
        psE = ctx.enter_context(tc.tile_pool(name=P("psE"), bufs=2, space="PSUM"))
        psW = ctx.enter_context(tc.tile_pool(name=P("psW"), bufs=2, space="PSUM"))

        xq_r = xq.rearrange("(p j q) d -> p j q d", j=2, q=128)
        out_r = out.rearrange("(p j q) d -> p j q d", j=2, q=128)

        for p in range(NCHUNK):
            xq_nat = [slot.tile([128, DIM], F32, tag=f"xqn{j}", name=f"xqn{j}") for j in range(2)]
            xqT = [s1.tile([128, QC], F32R, tag=f"xqT{i}", name=f"xqT{i}", bufs=2) for i in range(4)]
            for j in range(2):
                nc.sync.dma_start(xq_nat[j][:], xq_r[p, j])
                for i in range(4):
                    ps = psS.tile([128, 128], F32, tag="sT")
                    nc.tensor.transpose(ps[:], xq_nat[j][:, bass.ts(i, 128)], ident[:])
                    nc.vector.tensor_copy(xqT[i][:, bass.ts(j, 128)], ps[:])

            hTA = psH.tile([128, 2 * QC], F32, tag="hTA", name="hTA")
            hTB = psH.tile([128, 2 * QC], F32, tag="hTB", name="hTB")
            hT = [hTA[:, bass.ts(0, QC)], hTA[:, bass.ts(1, QC)],
                  hTB[:, bass.ts(0, QC)], hTB[:, bass.ts(1, QC)]]
            s_rep = psE.tile([128, QC], F32, tag="ff2")
            first_av = True
            for g in range(p + 1):
                if g < KCACHE_GROUPS:
                    kt_g, v_g = ktc[g], vc[g]
                else:
                    kt_g = [kstream.tile([128, 512], F32R, tag=f"kts{i}", name=f"kts{i}")
                            for i in range(4)]
                    v_g = [kstream.tile([128, 512], F32R, tag=f"vs{j}", name=f"vs{j}")
                           for j in range(4)]
                    for i in range(4):
                        nc.sync.dma_start(kt_g[i][:], kt_dram_r[i, :, bass.ts(g, 512)])
                    for j in range(4):
                        nc.sync.dma_start(v_g[j][:], v_dram_r[g, j])
                diag = (g == p)
                for j in range(4):
                    ps = psS.tile([128, QC], F32, tag="sT")
                    for i in range(4):
                        nc.tensor.matmul(ps[:], kt_g[i][:, bass.ts(j, 128)], xqT[i][:],
                                         start=(i == 0), stop=(i == 3))
                    pt = pt_pool.tile([128, QC], F32R, tag="pt")
                    if diag:
                        pe = pt_pool.tile([128, QC], F32R, tag="pe", bufs=2)
                        nc.scalar.activation(pe[:], ps[:], AF.Exp, scale=INV_SQRT_D)
                        nc.vector.tensor_tensor(pt[:], pe[:], cm[:, j, :], op=OP.mult)
                    else:
                        nc.scalar.activation(pt[:], ps[:], AF.Exp, scale=INV_SQRT_D)
                    last_av = (g == p) and (j == 3)
                    for i in range(4):
                        # start=True clears the whole PSUM bank, so only the
                        # first chain per packed bank may set it; the second
                        # chain writes into cleared has_written bits.
                        nc.tensor.matmul(hT[i], v_g[j][:, bass.ts(i, 128)], pt[:],
                                         start=first_av and (i % 2 == 0),
                                         stop=last_av, skip_group_check=True)
                    nc.tensor.matmul(s_rep[:], ones[:], pt[:],
                                     start=first_av, stop=last_av)
                    first_av = False

            r_bc = slot.tile([128, QC], F32, tag="r_bc")
            nc.vector.reciprocal(r_bc[:], s_rep[:])
            h_nrm = [s1.tile([128, QC], F32R, tag=f"hn{i}", name=f"hn{i}", bufs=2) for i in range(4)]
            for i in range(4):
                nc.vector.tensor_tensor(h_nrm[i][:], hT[i], r_bc[:], op=OP.mult)

            h_resT = [s1.tile([128, QC], F32R, tag=f"hrT{o}", name=f"hrT{o}", bufs=2) for o in range(4)]
            for o in range(4):
                ps = psW.tile([128, QC], F32, tag="wf")
                for i in range(4):
                    nc.tensor.matmul(ps[:], woT[i][:, bass.ts(o, 128)], h_nrm[i][:],
                                     start=(i == 0), stop=(i == 3))
                nc.vector.tensor_tensor(h_resT[o][:], ps[:], xqT[o][:], op=OP.add)
            h_resN = [slot.tile([128, DIM], F32, tag=f"hrN{j}", name=f"hrN{j}") for j in range(2)]
            for j in range(2):
                for o in range(4):
                    tp = psS.tile([128, 128], F32R, tag="sT", name="tph")
                    nc.tensor.transpose(tp[:], h_resT[o][:, bass.ts(j, 128)],
                                        ident_r[:])
                    nc.vector.tensor_copy(h_resN[j][:, bass.ts(o, 128)], tp[:])

            ff2 = [psE.tile([128, DIM], F32, tag="ff2", name=f"ff2_{j}") for j in range(2)]
            for hh in range(HID // 128):
                ps = psW.tile([128, QC], F32, tag="wf")
                for i in range(4):
                    nc.tensor.matmul(ps[:], w1T[i][:, bass.ts(hh, 128)], h_resT[i][:],
                                     start=(i == 0), stop=(i == 3))
                f1 = ff_pool.tile([128, QC], F32R, tag="f1")
                nc.scalar.activation(f1[:], ps[:], AF.Relu)
                for j in range(2):
                    nc.tensor.matmul(ff2[j][:], f1[:, bass.ts(j, 128)], w2T[hh][:],
                                     start=(hh == 0), stop=(hh == HID // 128 - 1))
            for j in range(2):
                ot = slot.tile([128, DIM], F32, tag=f"ot{j}")
                nc.vector.tensor_tensor(ot[:], ff2[j][:], h_resN[j][:], op=OP.add)
                nc.sync.dma_start(out_r[p, j], ot[:])

    nc.compile()
    return nc


def _shard_inputs(inputs):
    x = np.ascontiguousarray(inputs["x"], dtype=np.float32)
    in_maps = []
    for c in range(8):
        b, h = c // 2, c % 2
        xb = np.ascontiguousarray(x[b])
        xq = np.ascontiguousarray(
            xb.reshape(NCHUNK, 512, DIM)[:, QC * h:QC * h + QC, :].reshape(NQROWS, DIM))
        kr = np.arange(128)[None, :, None]
        qr = np.arange(QC)[None, None, :]
        jj = np.arange(4)[:, None, None]
        cmsk = (qr >= kr + 128 * jj - QC * h).astype(np.float32)
        m = {"xf": xb, "xq": xq, "cmask": np.ascontiguousarray(cmsk)}
        for k, v in inputs.items():
            if k not in ("x", "mask"):
                m[k] = np.ascontiguousarray(v, dtype=np.float32)
        in_maps.append(m)
    return in_maps


def kernel(**inputs):
    if "nc" not in _CACHE:
        _CACHE["nc"] = _build_nc()
    nc = _CACHE["nc"]
    in_maps = _shard_inputs(inputs)
    res = run_bass_kernel_spmd(nc, in_maps, core_ids=list(range(8)))
    out = np.empty((BS, SLEN, DIM), dtype=np.float32)
    for c in range(8):
        b, h = c // 2, c % 2
        o = res.results[c]["out"].reshape(NCHUNK, QC, DIM)
        out.reshape(BS, NCHUNK, 512, DIM)[b, :, QC * h:QC * h + QC, :] = o
    return out


# revision 10
# speedup vs baseline: 2.4646x; 1.2983x over previous
"""Bayesian transformer block on 8 trn2 cores.

Sharding: core c -> batch b=c//2, half h=c%2. Each core handles 8 query
chunks of 256 rows: chunk p = rows [512p+256h, 512p+256h+256) of batch b.
Static per-chunk key extent 512(p+1); the causal boundary is enforced by a
data-driven multiplicative mask on the trailing 4 key tiles.

All matmuls run in float32r (tf32-like, full PE rate); residual adds in
fp32. Attention uses transposed scores S^T[k,q] so the exp output feeds
P@V directly as the moving operand; row sums come from an all-ones
stationary matmul; normalization is a reciprocal broadcast multiply
before the output projection.
"""
import sys, os

for _p in ("/opt/trn_rl_repo", "/root/.axon_site/_ro/trn_rl_repo"):
    if os.path.isdir(_p) and _p not in sys.path:
        sys.path.insert(0, _p)

import numpy as np
from contextlib import ExitStack

import concourse.bass as bass
import concourse.bacc as bacc
import concourse.mybir as mybir
import concourse.tile as tile
from concourse.bass_utils import run_bass_kernel_spmd
from concourse.masks import make_identity

F32 = mybir.dt.float32
F32R = mybir.dt.float32r
AF = mybir.ActivationFunctionType
OP = mybir.AluOpType

DIM = 512
HID = 2048
BS, SLEN = 4, 4096
NCHUNK = 8          # query chunks per core, 256 rows each
QC = 256            # queries per chunk
NQROWS = NCHUNK * QC
KCACHE_GROUPS = 3   # 512-key groups kept resident in SBUF
NG = SLEN // 512
INV_SQRT_D = float(1.0 / np.sqrt(DIM))

_CACHE = {}


def _build_nc(reps=1):
    nc = bacc.Bacc("TRN2", target_bir_lowering=False, debug=False, num_devices=8,
                   dynamic_dma_scratch_size=2048)

    xf = nc.dram_tensor("xf", [SLEN, DIM], F32, kind="ExternalInput").ap()
    xq = nc.dram_tensor("xq", [NQROWS, DIM], F32, kind="ExternalInput").ap()
    cmask = nc.dram_tensor("cmask", [4, 128, QC], F32, kind="ExternalInput").ap()
    wio = {}
    for w, (o, i) in (("wk", (DIM, DIM)), ("wv", (DIM, DIM)), ("wo", (DIM, DIM)),
                      ("w1", (HID, DIM)), ("w2", (DIM, HID))):
        for sfx in ("mu", "ls", "eps"):
            wio[f"{w}_{sfx}"] = nc.dram_tensor(f"{w}_{sfx}", [o, i], F32,
                                               kind="ExternalInput").ap()
    out = nc.dram_tensor("out", [NQROWS, DIM], F32, kind="ExternalOutput").ap()

    kt_dram = nc.dram_tensor("kt_spill", [DIM, SLEN], F32R).ap()
    v_dram = nc.dram_tensor("v_spill", [SLEN, DIM], F32R).ap()

    with tile.TileContext(nc) as tc:
      for _rep in range(reps):
       with ExitStack() as ctx:
        P = lambda n: f"{n}_{_rep}"
        const = ctx.enter_context(tc.tile_pool(name=P("const"), bufs=1))
        wres = ctx.enter_context(tc.tile_pool(name=P("wres"), bufs=1))
        kvcache = ctx.enter_context(tc.tile_pool(name=P("kvcache"), bufs=1))

        ident = const.tile([128, 128], F32, tag="ident")
        make_identity(nc, ident[:])
        ones32 = const.tile([128, 128], F32, tag="ones32")
        nc.gpsimd.memset(ones32[:], 1.0)
        ones = const.tile([128, 128], F32R, tag="ones")
        nc.vector.tensor_copy(ones[:], ones32[:])
        ident_r = const.tile([128, 128], F32R, tag="ident_r")
        nc.vector.tensor_copy(ident_r[:], ident[:])
        cm = const.tile([128, 4, QC], F32, tag="cm")
        nc.sync.dma_start(cm[:], cmask.rearrange("j p q -> p j q"))

        def build_wT(w, o_dim, i_dim, dst_pool):
            """Perturb W = mu + exp(ls)*eps, return W^T as i_dim//128 tiles
            [128, o_dim] f32r (partition = input dim)."""
            wt = [dst_pool.tile([128, o_dim], F32R, tag=f"{w}T{i}", name=f"{w}T{i}")
                  for i in range(i_dim // 128)]
            mu_r = wio[f"{w}_mu"].rearrange("(a p) i -> a p i", p=128)
            ls_r = wio[f"{w}_ls"].rearrange("(a p) i -> a p i", p=128)
            ep_r = wio[f"{w}_eps"].rearrange("(a p) i -> a p i", p=128)
            IC = min(i_dim, 512)
            with ExitStack() as stk:
                stage = stk.enter_context(tc.tile_pool(name=P(f"stg_{w}"), bufs=2))
                pst = stk.enter_context(
                    tc.tile_pool(name=P(f"pst_{w}"), bufs=2, space="PSUM"))
                for a in range(o_dim // 128):
                    for cb in range(i_dim // IC):
                        mu = stage.tile([128, IC], F32, tag="mu")
                        ls = stage.tile([128, IC], F32, tag="ls")
                        ep = stage.tile([128, IC], F32, tag="ep")
                        nc.sync.dma_start(mu[:], mu_r[a][:, bass.ts(cb, IC)])
                        nc.sync.dma_start(ls[:], ls_r[a][:, bass.ts(cb, IC)])
                        nc.sync.dma_start(ep[:], ep_r[a][:, bass.ts(cb, IC)])
                        els = stage.tile([128, IC], F32, tag="els")
                        nc.scalar.activation(els[:], ls[:], AF.Exp)
                        prod = stage.tile([128, IC], F32, tag="prod")
                        nc.gpsimd.tensor_tensor(prod[:], els[:], ep[:], op=OP.mult)
                        wnat = stage.tile([128, IC], F32, tag="wnat")
                        nc.vector.tensor_tensor(wnat[:], prod[:], mu[:], op=OP.add)
                        for ii in range(IC // 128):
                            i = cb * (IC // 128) + ii
                            ps = pst.tile([128, 128], F32, tag="tp")
                            nc.tensor.transpose(ps[:], wnat[:, bass.ts(ii, 128)],
                                                ident[:])
                            nc.vector.tensor_copy(wt[i][:, bass.ts(a, 128)], ps[:])
            return wt

        # ---- K^T / V over all 4096 keys (cache 3 groups, spill the rest) ----
        xf_r = xf.rearrange("(g j p) d -> g j p d", j=4, p=128)
        kt_dram_r = kt_dram.rearrange("(t p) s -> t p s", p=128)
        v_dram_r = v_dram.rearrange("(g j p) d -> g j p d", j=4, p=128)

        ktc = [[kvcache.tile([128, 512], F32R, tag=f"ktc{g}_{i}", name=f"ktc{g}_{i}")
                for i in range(4)] for g in range(KCACHE_GROUPS)]
        vc = [[kvcache.tile([128, 512], F32R, tag=f"vc{g}_{j}", name=f"vc{g}_{j}")
               for j in range(4)] for g in range(KCACHE_GROUPS)]

        with ExitStack() as stk:
            wkv = stk.enter_context(tc.tile_pool(name=P("wkv"), bufs=1))
            wkT = build_wT("wk", DIM, DIM, wkv)
            wvT = build_wT("wv", DIM, DIM, wkv)
            stage = stk.enter_context(tc.tile_pool(name=P("stg_x"), bufs=3))
            spill = stk.enter_context(tc.tile_pool(name=P("spill"), bufs=3))
            pstx = stk.enter_context(tc.tile_pool(name=P("pst_x"), bufs=2, space="PSUM"))
            psb = stk.enter_context(tc.tile_pool(name=P("psB"), bufs=2, space="PSUM"))
            for g in range(NG):
                xfT = [stage.tile([128, 512], F32R, tag=f"xfT{i}", name=f"xfT{i}") for i in range(4)]
                for j in range(4):
                    xt = stage.tile([128, DIM], F32, tag="xrow")
                    nc.sync.dma_start(xt[:], xf_r[g, j])
                    for i in range(4):
                        ps = pstx.tile([128, 128], F32, tag="tp")
                        nc.tensor.transpose(ps[:], xt[:, bass.ts(i, 128)], ident[:])
                        nc.vector.tensor_copy(xfT[i][:, bass.ts(j, 128)], ps[:])
                for o in range(4):
                    ps = psb.tile([128, 512], F32, tag="kps")
                    for i in range(4):
                        nc.tensor.matmul(ps[:], wkT[i][:, bass.ts(o, 128)], xfT[i][:],
                                         start=(i == 0), stop=(i == 3))
                    if g < KCACHE_GROUPS:
                        nc.scalar.copy(ktc[g][o][:], ps[:])
                    else:
                        sp = spill.tile([128, 512], F32R, tag="ksp")
                        nc.scalar.copy(sp[:], ps[:])
                        nc.sync.dma_start(kt_dram_r[o, :, bass.ts(g, 512)], sp[:])
                for j in range(4):
                    ps = psb.tile([128, 512], F32, tag="vps")
                    for i in range(4):
                        nc.tensor.matmul(ps[:], xfT[i][:, bass.ts(j, 128)], wvT[i][:],
                                         start=(i == 0), stop=(i == 3))
                    if g < KCACHE_GROUPS:
                        nc.vector.tensor_copy(vc[g][j][:], ps[:])
                    else:
                        sp = spill.tile([128, 512], F32R, tag="vsp")
                        nc.vector.tensor_copy(sp[:], ps[:])
                        nc.sync.dma_start(v_dram_r[g, j], sp[:])

        woT = build_wT("wo", DIM, DIM, wres)
        w1T = build_wT("w1", HID, DIM, wres)
        w2T = build_wT("w2", DIM, HID, wres)

        # ---- per-slot attention + FFN ----
        slot = ctx.enter_context(tc.tile_pool(name=P("slot"), bufs=2))
        s1 = ctx.enter_context(tc.tile_pool(name=P("s1"), bufs=1))
        pt_pool = ctx.enter_context(tc.tile_pool(name=P("pt"), bufs=3))
        ff_pool = ctx.enter_context(tc.tile_pool(name=P("ff"), bufs=3))
        kstream = ctx.enter_context(tc.tile_pool(name=P("kstream"), bufs=3))
        # PSUM budget (8 banks): psS 2 ("sT": transposes+scores) +
        # psH 2 (packed AV accumulators) + psE 2 ("ff2": s_rep & ff2_0/1) +
        # psW 2 ("wf": woT then ff1)
        psS = ctx.enter_context(tc.tile_pool(name=P("psS"), bufs=2, space="PSUM"))
        psH = ctx.enter_context(tc.tile_pool(name=P("psH"), bufs=1, space="PSUM"))
        psE = ctx.enter_context(tc.tile_pool(name=P("psE"), bufs=2, space="PSUM"))
        psW = ctx.enter_context(tc.tile_pool(name=P("psW"), bufs=2, space="PSUM"))

        xq_r = xq.rearrange("(p j q) d -> p j q d", j=2, q=128)
        out_r = out.rearrange("(p j q) d -> p j q d", j=2, q=128)

        for p in range(NCHUNK):
            xq_nat = [slot.tile([128, DIM], F32, tag=f"xqn{j}", name=f"xqn{j}") for j in range(2)]
            xqT = [s1.tile([128, QC], F32R, tag=f"xqT{i}", name=f"xqT{i}", bufs=2) for i in range(4)]
            for j in range(2):
                nc.sync.dma_start(xq_nat[j][:], xq_r[p, j])
                for i in range(4):
                    ps = psS.tile([128, 128], F32, tag="sT")
                    nc.tensor.transpose(ps[:], xq_nat[j][:, bass.ts(i, 128)], ident[:])
                    nc.vector.tensor_copy(xqT[i][:, bass.ts(j, 128)], ps[:])

            hTA = psH.tile([128, 2 * QC], F32, tag="hTA", name="hTA")
            hTB = psH.tile([128, 2 * QC], F32, tag="hTB", name="hTB")
            hT = [hTA[:, bass.ts(0, QC)], hTA[:, bass.ts(1, QC)],
                  hTB[:, bass.ts(0, QC)], hTB[:, bass.ts(1, QC)]]
            s_rep = psE.tile([128, QC], F32, tag="ff2")
            first_av = True
            for g in range(p + 1):
                if g < KCACHE_GROUPS:
                    kt_g, v_g = ktc[g], vc[g]
                else:
                    kt_g = [kstream.tile([128, 512], F32R, tag=f"kts{i}", name=f"kts{i}")
                            for i in range(4)]
                    v_g = [kstream.tile([128, 512], F32R, tag=f"vs{j}", name=f"vs{j}", bufs=2)
                           for j in range(4)]
                    for i in range(4):
                        nc.sync.dma_start(kt_g[i][:], kt_dram_r[i, :, bass.ts(g, 512)])
                    for j in range(4):
                        nc.sync.dma_start(v_g[j][:], v_dram_r[g, j])
                diag = (g == p)
                for j in range(4):
                    ps = psS.tile([128, QC], F32, tag="sT")
                    for i in range(4):
                        nc.tensor.matmul(ps[:], kt_g[i][:, bass.ts(j, 128)], xqT[i][:],
                                         start=(i == 0), stop=(i == 3))
                    pt = pt_pool.tile([128, QC], F32R, tag="pt")
                    if diag:
                        pe = pt_pool.tile([128, QC], F32R, tag="pe", bufs=2)
                        nc.scalar.activation(pe[:], ps[:], AF.Exp, scale=INV_SQRT_D)
                        nc.vector.tensor_tensor(pt[:], pe[:], cm[:, j, :], op=OP.mult)
                    else:
                        nc.scalar.activation(pt[:], ps[:], AF.Exp, scale=INV_SQRT_D)
                    last_av = (g == p) and (j == 3)
                    for i in range(4):
                        # start=True clears the whole PSUM bank, so only the
                        # first chain per packed bank may set it; the second
                        # chain writes into cleared has_written bits.
                        nc.tensor.matmul(hT[i], v_g[j][:, bass.ts(i, 128)], pt[:],
                                         start=first_av and (i % 2 == 0),
                                         stop=last_av, skip_group_check=True)
                    nc.tensor.matmul(s_rep[:], ones[:], pt[:],
                                     start=first_av, stop=last_av)
                    first_av = False

            r_bc = slot.tile([128, QC], F32, tag="r_bc")
            nc.vector.reciprocal(r_bc[:], s_rep[:])
            h_nrm = [s1.tile([128, QC], F32R, tag=f"hn{i}", name=f"hn{i}", bufs=2) for i in range(4)]
            for i in range(4):
                nc.vector.tensor_tensor(h_nrm[i][:], hT[i], r_bc[:], op=OP.mult)

            h_resT = [s1.tile([128, QC], F32R, tag=f"hrT{o}", name=f"hrT{o}", bufs=2) for o in range(4)]
            for o in range(4):
                ps = psW.tile([128, QC], F32, tag="wf")
                for i in range(4):
                    nc.tensor.matmul(ps[:], woT[i][:, bass.ts(o, 128)], h_nrm[i][:],
                                     start=(i == 0), stop=(i == 3))
                nc.vector.tensor_tensor(h_resT[o][:], ps[:], xqT[o][:], op=OP.add)
            h_resN = [slot.tile([128, DIM], F32, tag=f"hrN{j}", name=f"hrN{j}", bufs=1) for j in range(2)]
            for j in range(2):
                for o in range(4):
                    tp = psS.tile([128, 128], F32R, tag="sT", name="tph")
                    nc.tensor.transpose(tp[:], h_resT[o][:, bass.ts(j, 128)],
                                        ident_r[:])
                    nc.vector.tensor_copy(h_resN[j][:, bass.ts(o, 128)], tp[:])

            ff2 = [psE.tile([128, DIM], F32, tag="ff2", name=f"ff2_{j}") for j in range(2)]
            for hh in range(HID // 128):
                ps = psW.tile([128, QC], F32, tag="wf")
                for i in range(4):
                    nc.tensor.matmul(ps[:], w1T[i][:, bass.ts(hh, 128)], h_resT[i][:],
                                     start=(i == 0), stop=(i == 3))
                f1 = ff_pool.tile([128, QC], F32R, tag="f1")
                nc.scalar.activation(f1[:], ps[:], AF.Relu)
                for j in range(2):
                    nc.tensor.matmul(ff2[j][:], f1[:, bass.ts(j, 128)], w2T[hh][:],
                                     start=(hh == 0), stop=(hh == HID // 128 - 1))
            for j in range(2):
                ot = slot.tile([128, DIM], F32, tag=f"ot{j}")
                nc.vector.tensor_tensor(ot[:], ff2[j][:], h_resN[j][:], op=OP.add)
                nc.sync.dma_start(out_r[p, j], ot[:])

    nc.compile()
    return nc


def _shard_inputs(inputs):
    x = np.ascontiguousarray(inputs["x"], dtype=np.float32)
    in_maps = []
    for c in range(8):
        b, h = c // 2, c % 2
        xb = np.ascontiguousarray(x[b])
        xq = np.ascontiguousarray(
            xb.reshape(NCHUNK, 512, DIM)[:, QC * h:QC * h + QC, :].reshape(NQROWS, DIM))
        kr = np.arange(128)[None, :, None]
        qr = np.arange(QC)[None, None, :]
        jj = np.arange(4)[:, None, None]
        cmsk = (qr >= kr + 128 * jj - QC * h).astype(np.float32)
        m = {"xf": xb, "xq": xq, "cmask": np.ascontiguousarray(cmsk)}
        for k, v in inputs.items():
            if k not in ("x", "mask"):
                m[k] = np.ascontiguousarray(v, dtype=np.float32)
        in_maps.append(m)
    return in_maps


def kernel(**inputs):
    if "nc" not in _CACHE:
        _CACHE["nc"] = _build_nc()
    nc = _CACHE["nc"]
    in_maps = _shard_inputs(inputs)
    res = run_bass_kernel_spmd(nc, in_maps, core_ids=list(range(8)))
    out = np.empty((BS, SLEN, DIM), dtype=np.float32)
    for c in range(8):
        b, h = c // 2, c % 2
        o = res.results[c]["out"].reshape(NCHUNK, QC, DIM)
        out.reshape(BS, NCHUNK, 512, DIM)[b, :, QC * h:QC * h + QC, :] = o
    return out
